# revision 8
# baseline (speedup 1.0000x reference)
"""AutoCorrelation Trainium2 kernel (v3).

Reference reformulation (verified to 3e-7 rel):
  H=8, L=2048, D=512, k_sel=4, SCALE=1/(H*L)
  qbar = sum_l queries[b,l,:]
  mc = qbar @ wqk @ keys^T           (wqk = wq @ wk^T, host-precomputed)
  top_idx = top4(mc); w = softmax(SCALE * top_vals)
  VpT[c, l] = (values[b] @ wv_half)^T
  AwT[c, l] = sum_j w_j VpT[c, (l + idx_j) mod L]
  out rows (reference transpose quirk): for each r in 0..3:
      out_rows(r) = Aw[r*512:(r+1)*512, :].T @ wo

Sharding: 8 cores = 4 batches x 2 channel-halves; per-batch front-end
computed redundantly on the half-pair.

v3 structure (per core):
  - PE warm-up: chained dummy matmuls keep HAM at 8/8 through the DMA wait
  - qbar: DVE tensor_reduce (qt tiles 0,1) + ACT activation accum_out (2,3)
  - g2col (16 N=1 matmuls), mc row (16 matmuls, fp8 kt rhs)
  - top-8 DVE max/max_index; softmax; w broadcast via GpSimd
  - VpT on PE (fp16), l-doubling copies on GpSimd
  - AwT: per (r, ct) PSUM group of 4 identity matmuls whose rhs are
    dynamic-register slices of doubled VpT (roll fused into the matmul;
    weights folded into w_j-scaled identities built by ACT)
  - PE-transpose AwT tiles -> Aw, final GEMM Aw_r^T @ wo, fp32 out
Dtypes: q fp16, k fp8e4m3 (selection margin ~1000x rounding noise),
v/weights/intermediates fp16, all accumulation fp32.
"""

import numpy as np

B, L, D = 4, 2048, 512
H = 8
K_SEL = 4
SCALE = 1.0 / (H * L)
N_CORES = 8
P = 128
CH = 256          # channels per core (half of 512)
DK = D // P       # 4 d-tiles
N_WARM = 36


def _build_nc():
    import concourse.bass as bass
    import concourse.bacc as bacc
    import concourse.mybir as mybir
    from concourse.tile import TileContext
    from concourse.masks import make_identity
    from concourse import bass_isa

    fp32 = mybir.dt.float32
    f16 = mybir.dt.float16
    f8 = mybir.dt.float8e4
    u32 = mybir.dt.uint32
    i32 = mybir.dt.int32
    AX = mybir.AxisListType.X
    MUL = mybir.AluOpType.mult
    Copy = mybir.ActivationFunctionType.Copy
    Exp = mybir.ActivationFunctionType.Exp

    nc = bacc.Bacc("TRN2", target_bir_lowering=False, debug=False, num_devices=N_CORES)

    qt_dram = nc.dram_tensor("qt", [D, L], f16, kind="ExternalInput")    # queries^T
    kt_dram = nc.dram_tensor("kt", [D, L], f8, kind="ExternalInput")     # keys^T
    vt_dram = nc.dram_tensor("vt", [D, L], f16, kind="ExternalInput")    # values^T
    wqk_dram = nc.dram_tensor("wqk", [D, D], f16, kind="ExternalInput")  # wq @ wk^T
    wvh_dram = nc.dram_tensor("wvh", [D, CH], f16, kind="ExternalInput")
    wo_dram = nc.dram_tensor("wo", [D, D], f16, kind="ExternalInput")
    out_dram = nc.dram_tensor("out", [L // 2, D], fp32, kind="ExternalOutput")

    with TileContext(nc) as tc:
        with (
            tc.tile_pool(name="const", bufs=1) as cpool,
            tc.tile_pool(name="wts", bufs=1) as wts,
            tc.tile_pool(name="big", bufs=1) as big,
            tc.tile_pool(name="stream", bufs=2) as stream,
            tc.tile_pool(name="small", bufs=1) as small,
            tc.tile_pool(name="ps_warm", bufs=1, space="PSUM") as ps_warm,
            tc.tile_pool(name="ps_fe", bufs=2, space="PSUM") as ps_fe,
            tc.tile_pool(name="ps_mm", bufs=3, space="PSUM") as ps_mm,
            tc.tile_pool(name="ps_tp", bufs=2, space="PSUM") as ps_tp,
        ):
            ident = cpool.tile([P, P], fp32, tag="ident")
            make_identity(nc, ident)
            ident16 = cpool.tile([P, P], f16, tag="ident16")
            nc.scalar.copy(ident16, ident)
            zscr = cpool.tile([P, D], f16, tag="zscr")
            nc.gpsimd.memset(zscr, 0.0)

            # ---- PE warm-up chain (keeps HAM un-throttled through DMA wait) ----
            warm = ps_warm.tile([P, D], fp32, tag="warm")
            for i in range(N_WARM):
                nc.tensor.matmul(warm, ident16, zscr,
                                 start=(i == 0), stop=(i == N_WARM - 1))

            # ---- SBUF input tiles ----
            qt01_sb = big.tile([P, 2, L], f16, tag="qt01", name="qt01")
            qt2_sb = big.tile([P, L], f16, tag="qt2", name="qt2")
            qt3_sb = big.tile([P, L], f16, tag="qt3", name="qt3")
            kt_sb = big.tile([P, DK, L], f8, tag="kt", name="kt")
            vt01_sb = big.tile([P, 2, L], f16, tag="vt01", name="vt01")
            vt23_sb = big.tile([P, 2, L], f16, tag="vt23", name="vt23")
            wqk_sb = wts.tile([P, DK, D], f16, tag="wqk", name="wqk")
            wvh_sb = wts.tile([P, DK, CH], f16, tag="wvh", name="wvh")
            wo_sb = wts.tile([P, DK, D], f16, tag="wo", name="wo")

            qt_v = qt_dram.rearrange("(t p) l -> p t l", p=P)
            kt_v = kt_dram.rearrange("(t p) l -> p t l", p=P)
            vt_v = vt_dram.rearrange("(t p) l -> p t l", p=P)
            wqk_v = wqk_dram.rearrange("(t p) d -> p t d", p=P)
            wvh_v = wvh_dram.rearrange("(t p) c -> p t c", p=P)
            wo_v = wo_dram.rearrange("(t p) d -> p t d", p=P)

            # sync: qt01 wqk vt01 wo | scalar: wvh kt qt2 qt3 vt23
            nc.sync.dma_start(qt01_sb, qt_v[:, 0:2])
            nc.scalar.dma_start(wvh_sb, wvh_v)
            nc.scalar.dma_start(kt_sb, kt_v)
            nc.sync.dma_start(wqk_sb, wqk_v)
            nc.scalar.dma_start(qt2_sb, qt_v[:, 2])
            nc.sync.dma_start(vt01_sb, vt_v[:, 0:2])
            nc.scalar.dma_start(qt3_sb, qt_v[:, 3])
            nc.scalar.dma_start(vt23_sb, vt_v[:, 2:4])
            nc.sync.dma_start(wo_sb, wo_v)

            # ---- qbar: DVE reduce (tiles 0,1) + ACT accum (tiles 2,3) ----
            awT = big.tile([P, 2, L], f16, tag="awT", name="awT")
            qbcol = small.tile([P, DK], fp32, tag="qbcol")
            nc.vector.reduce_sum(qbcol[:, 0:2], qt01_sb, axis=AX)
            nc.scalar.activation(awT[:, 0, :], qt2_sb, Copy,
                                 accum_out=qbcol[:, 2:3])
            nc.scalar.activation(awT[:, 1, :], qt3_sb, Copy,
                                 accum_out=qbcol[:, 3:4])
            qb16 = small.tile([P, DK], f16, tag="qb16")
            nc.scalar.copy(qb16, qbcol)

            # ---- g2col = (wqk^T @ qbar) column chunks [128, 4] ----
            g2c16 = small.tile([P, DK], f16, tag="g2c16")
            for m in range(DK):
                psg = ps_mm.tile([P, D], fp32, tag="mm")
                for kk in range(DK):
                    nc.tensor.matmul(
                        psg[:, 0:1], wqk_sb[:, kk, m * P:(m + 1) * P],
                        qb16[:, kk:kk + 1],
                        start=(kk == 0), stop=(kk == DK - 1),
                    )
                nc.scalar.copy(g2c16[:, m:m + 1], psg[:, 0:1])

            # ---- mc row [1, 2048] = g2 @ keys^T ----
            mc_flat = small.tile([1, L], fp32, tag="mc_flat")
            for nch in range(4):
                psm = ps_fe.tile([1, 512], fp32, tag="mc")
                for dk in range(DK):
                    nc.tensor.matmul(
                        psm, g2c16[:, dk:dk + 1],
                        kt_sb[:, dk, nch * 512:(nch + 1) * 512],
                        start=(dk == 0), stop=(dk == DK - 1),
                    )
                nc.scalar.copy(mc_flat[0:1, nch * 512:(nch + 1) * 512], psm)

            # ---- VpT = wvh^T @ vt (before top-k in ACT/PE queue order) ----
            vpT = big.tile([P, 2, 2 * L], f16, tag="vpT", name="vpT")
            for ct in range(2):
                for lc in range(4):
                    pv = ps_mm.tile([P, 512], fp32, tag="mm")
                    for dk in range(DK):
                        vsrc = vt01_sb if dk < 2 else vt23_sb
                        nc.tensor.matmul(
                            pv,
                            wvh_sb[:, dk, ct * P:(ct + 1) * P],
                            vsrc[:, dk % 2, lc * 512:(lc + 1) * 512],
                            start=(dk == 0), stop=(dk == DK - 1),
                        )
                    nc.scalar.copy(vpT[:, ct, lc * 512:(lc + 1) * 512], pv)
                nc.gpsimd.tensor_copy(vpT[:, ct, L:2 * L], vpT[:, ct, 0:L])

            # ---- top-8 + softmax over top-4 ----
            mx8 = small.tile([1, 8], fp32, tag="mx8")
            mi8 = small.tile([1, 8], u32, tag="mi8")
            nc.vector.max(out=mx8, in_=mc_flat)
            nc.vector.max_index(out=mi8, in_max=mx8, in_values=mc_flat)
            e4 = small.tile([1, K_SEL], fp32, tag="e4")
            nc.scalar.activation(e4, mx8[0:1, 0:K_SEL], Exp, scale=float(SCALE))
            s1 = small.tile([1, 1], fp32, tag="s1")
            nc.vector.reduce_sum(s1, e4, axis=AX)
            r1 = small.tile([1, 1], fp32, tag="r1")
            nc.vector.reciprocal(r1, s1)
            w4 = small.tile([1, K_SEL], fp32, tag="w4")
            nc.vector.tensor_scalar(w4, e4, r1[0:1, 0:1], None, op0=MUL)
            wb = small.tile([P, K_SEL], fp32, tag="wb_sb")
            nc.gpsimd.partition_broadcast(wb, w4)

            # w_j-scaled identities (ACT per-partition scaled copies)
            wjI = [small.tile([P, P], f16, tag=f"wjI{j}", name=f"wjI{j}")
                   for j in range(K_SEL)]
            for j in range(K_SEL):
                nc.scalar.activation(wjI[j], ident16, Copy, scale=wb[:, j:j + 1])

            s_regs = [
                nc.values_load(
                    mi8[0:1, j:j + 1].bitcast(i32),
                    engines=(mybir.EngineType.PE,),
                    min_val=0, max_val=L - 1,
                    skip_runtime_bounds_check=True,
                ) for j in range(K_SEL)
            ]

            # ---- AwT per (r, ct): 4 dyn-slice identity matmuls; transpose;
            #      final GEMM per (r, cm=ct) ----
            for r in range(4):
                aw = [small.tile([P, CH], f16, tag=f"aw_{r}_{lp}", name=f"aw{r}_{lp}")
                      for lp in range(4)]
                for ct in range(2):
                    pa = ps_mm.tile([P, 512], fp32, tag="mm")
                    for j in range(K_SEL):
                        nc.tensor.matmul(
                            pa, wjI[j],
                            vpT[:, ct, bass.ds(s_regs[j] + r * 512, 512)],
                            start=(j == 0), stop=(j == K_SEL - 1),
                        )
                    awTs = awT[:, ct, r * 512:(r + 1) * 512]
                    nc.scalar.copy(awTs, pa)
                    for lp in range(4):
                        pt = ps_tp.tile([P, P], f16, tag="tp")
                        nc.tensor.transpose(
                            pt, awTs[:, lp * P:(lp + 1) * P], ident16)
                        if lp % 2 == 0:
                            nc.scalar.copy(aw[lp][:, ct * P:(ct + 1) * P], pt)
                        else:
                            nc.vector.tensor_copy(aw[lp][:, ct * P:(ct + 1) * P], pt)
                    po = ps_mm.tile([P, D], fp32, tag="mm")
                    for lp in range(4):
                        nc.tensor.matmul(
                            po, aw[lp][:, ct * P:(ct + 1) * P], wo_sb[:, lp],
                            start=(lp == 0), stop=(lp == DK - 1),
                        )
                    ot = stream.tile([P, D], fp32, tag="otile")
                    if ct % 2 == 0:
                        nc.scalar.copy(ot, po)
                    else:
                        nc.vector.tensor_copy(ot, po)
                    row0 = r * 256 + ct * P
                    eng = nc.sync if ct == 0 else nc.scalar
                    eng.dma_start(out_dram[row0:row0 + P, :], ot)

    nc.compile()
    return nc


_NC_CACHE = None


def _get_nc():
    global _NC_CACHE
    if _NC_CACHE is None:
        _NC_CACHE = _build_nc()
    return _NC_CACHE


def _half_cols(half):
    d0 = 32 * half
    return np.array([(cl // 32) * 64 + d0 + cl % 32 for cl in range(CH)])


def _row_index(half):
    # device row r*256 + cl  ->  full-output row i
    d0 = 32 * half
    idx = np.empty(1024, np.int64)
    for r in range(4):
        for cl in range(CH):
            i = (d0 + cl % 32) * 32 + (cl // 32) * 4 + r
            idx[r * CH + cl] = i
    return idx


def make_in_maps(queries, keys, values, wq, wk, wv, wo):
    import ml_dtypes
    f8 = ml_dtypes.float8_e4m3
    wqk = (wq.astype(np.float64) @ wk.T.astype(np.float64)).astype(np.float16)
    wo16 = wo.astype(np.float16)
    in_maps = []
    for c in range(N_CORES):
        b, half = c // 2, c % 2
        qt = np.ascontiguousarray(queries[b].T).astype(np.float16)
        kt = np.ascontiguousarray(keys[b].T).astype(f8)
        vt = np.ascontiguousarray(values[b].T).astype(np.float16)
        wvh = np.ascontiguousarray(wv[:, _half_cols(half)]).astype(np.float16)
        in_maps.append({
            "qt": qt, "kt": kt, "vt": vt,
            "wqk": wqk, "wvh": wvh, "wo": wo16,
        })
    return in_maps


def kernel(queries, keys, values, wq, wk, wv, wo, trace=False):
    import sys
    if "/opt/trn_rl_repo" not in sys.path:
        sys.path.insert(0, "/opt/trn_rl_repo")
    from concourse import bass_utils

    nc = _get_nc()
    in_maps = make_in_maps(queries, keys, values, wq, wk, wv, wo)
    res = bass_utils.run_bass_kernel_spmd(
        nc, in_maps, core_ids=list(range(N_CORES)), trace=trace,
    )
    out = np.empty((B, L, D), np.float32)
    for c in range(N_CORES):
        b, half = c // 2, c % 2
        out[b, _row_index(half), :] = res.results[c]["out"]
    if trace:
        return out, res
    return out


# revision 9
# speedup vs baseline: 1.0342x; 1.0342x over previous
"""AutoCorrelation Trainium2 kernel (v4).

Reference reformulation (verified to 3e-7 rel):
  H=8, L=2048, D=512, k_sel=4, SCALE=1/(H*L)
  qbar = sum_l queries[b,l,:]
  mc = qbar @ wqk @ keys^T           (wqk = wq @ wk^T, host-precomputed)
  top_idx = top4(mc); w = softmax(SCALE * top_vals)
  VpT[c, l] = (values[b] @ wv_half)^T
  AwT[c, l] = sum_j w_j VpT[c, (l + idx_j) mod L]
  out rows (reference transpose quirk): for each r in 0..3:
      out_rows(r) = Aw[r*512:(r+1)*512, :].T @ wo

Sharding: 8 cores = 4 batches x 2 channel-halves; per-batch front-end
computed redundantly on the half-pair.

v4 (per core), engine-FIFO-aware:
  - PE warm-up chain keeps HAM at 8/8 through the DMA wait
  - qbar: 2 DVE reduces + 2 ACT accum_out activations, per-tile as DMAs land
  - g2col (16 N=1 matmuls), mc row (16 matmuls, fp8 kt rhs), top-8 DVE
  - VpT on PE, vt loaded as 4 l-chunks so the GEMM pipelines with DMA;
    l-doubling via 2 DVE copies; w broadcast GpSimd; w-scaled identities ACT
  - AwT per (r, ct): 4 identity matmuls with dynamic-register rhs slices of
    doubled VpT (roll + weighted sum fused into PE accumulation)
  - 4 transposes per (r, ct) into ONE psum bank -> 1 copy; final GEMM; out
Dtypes: q fp16, k fp8e4m3 (selection margin ~1000x rounding noise),
v/weights/intermediates fp16, accumulation fp32.
"""

import numpy as np

B, L, D = 4, 2048, 512
H = 8
K_SEL = 4
SCALE = 1.0 / (H * L)
N_CORES = 8
P = 128
CH = 256          # channels per core (half of 512)
DK = D // P       # 4 d-tiles
N_WARM = 50


def _build_nc():
    import concourse.bass as bass
    import concourse.bacc as bacc
    import concourse.mybir as mybir
    from concourse.tile import TileContext
    from concourse.masks import make_identity

    fp32 = mybir.dt.float32
    f16 = mybir.dt.float16
    f8 = mybir.dt.float8e4
    u32 = mybir.dt.uint32
    i32 = mybir.dt.int32
    AX = mybir.AxisListType.X
    MUL = mybir.AluOpType.mult
    Copy = mybir.ActivationFunctionType.Copy
    Exp = mybir.ActivationFunctionType.Exp

    nc = bacc.Bacc("TRN2", target_bir_lowering=False, debug=False, num_devices=N_CORES)

    qt_dram = nc.dram_tensor("qt", [D, L], f16, kind="ExternalInput")    # queries^T
    kt_dram = nc.dram_tensor("kt", [D, L], f8, kind="ExternalInput")     # keys^T
    vt_dram = nc.dram_tensor("vt", [D, L], f16, kind="ExternalInput")    # values^T
    wqk_dram = nc.dram_tensor("wqk", [D, D], f16, kind="ExternalInput")  # wq @ wk^T
    wvh_dram = nc.dram_tensor("wvh", [D, CH], f16, kind="ExternalInput")
    wo_dram = nc.dram_tensor("wo", [D, D], f16, kind="ExternalInput")
    out_dram = nc.dram_tensor("out", [L // 2, D], fp32, kind="ExternalOutput")

    with TileContext(nc) as tc:
        with (
            tc.tile_pool(name="const", bufs=1) as cpool,
            tc.tile_pool(name="wts", bufs=1) as wts,
            tc.tile_pool(name="big", bufs=1) as big,
            tc.tile_pool(name="stream", bufs=2) as stream,
            tc.tile_pool(name="small", bufs=1) as small,
            tc.tile_pool(name="ps_warm", bufs=1, space="PSUM") as ps_warm,
            tc.tile_pool(name="ps_fe", bufs=2, space="PSUM") as ps_fe,
            tc.tile_pool(name="ps_mm", bufs=3, space="PSUM") as ps_mm,
            tc.tile_pool(name="ps_tp", bufs=2, space="PSUM") as ps_tp,
        ):
            ident = cpool.tile([P, P], fp32, tag="ident")
            make_identity(nc, ident)
            ident16 = cpool.tile([P, P], f16, tag="ident16")
            nc.scalar.copy(ident16, ident)
            zscr = cpool.tile([P, D], f16, tag="zscr")
            nc.gpsimd.memset(zscr, 0.0)

            # ---- PE warm-up chain ----
            warm = ps_warm.tile([P, D], fp32, tag="warm")
            for i in range(N_WARM):
                nc.tensor.matmul(warm, ident16, zscr,
                                 start=(i == 0), stop=(i == N_WARM - 1))

            # ---- SBUF input tiles ----
            qt_sb = [big.tile([P, L], f16, tag=f"qt{i}", name=f"qt{i}")
                     for i in range(DK)]
            kt_sb = big.tile([P, DK, L], f8, tag="kt", name="kt")
            vtq = [big.tile([P, DK, 512], f16, tag=f"vt{i}", name=f"vt{i}")
                   for i in range(4)]
            wqk_sb = wts.tile([P, DK, D], f16, tag="wqk", name="wqk")
            wvh_sb = wts.tile([P, DK, CH], f16, tag="wvh", name="wvh")
            wo_sb = wts.tile([P, DK, D], f16, tag="wo", name="wo")

            qt_v = qt_dram.rearrange("(t p) l -> p t l", p=P)
            kt_v = kt_dram.rearrange("(t p) l -> p t l", p=P)
            vt_v = vt_dram.rearrange("(t p) l -> p t l", p=P)
            wqk_v = wqk_dram.rearrange("(t p) d -> p t d", p=P)
            wvh_v = wvh_dram.rearrange("(t p) c -> p t c", p=P)
            wo_v = wo_dram.rearrange("(t p) d -> p t d", p=P)

            # sync: qt0 qt1 wqk vtA vtB | scalar: qt2 qt3 kt wvh vtC vtD wo
            nc.sync.dma_start(qt_sb[0], qt_v[:, 0])
            nc.scalar.dma_start(qt_sb[2], qt_v[:, 2])
            nc.sync.dma_start(qt_sb[1], qt_v[:, 1])
            nc.scalar.dma_start(qt_sb[3], qt_v[:, 3])
            nc.sync.dma_start(wqk_sb, wqk_v)
            nc.scalar.dma_start(kt_sb, kt_v)
            nc.scalar.dma_start(wvh_sb, wvh_v)
            nc.sync.dma_start(vtq[0], vt_v[:, :, 0:512])
            nc.scalar.dma_start(vtq[2], vt_v[:, :, 1024:1536])
            nc.sync.dma_start(vtq[1], vt_v[:, :, 512:1024])
            nc.scalar.dma_start(vtq[3], vt_v[:, :, 1536:2048])
            nc.scalar.dma_start(wo_sb, wo_v)

            # ---- qbar: 2 DVE reduces + 2 ACT accum activations ----
            awT = big.tile([P, 2, L], f16, tag="awT", name="awT")
            qbcol = small.tile([P, DK], fp32, tag="qbcol")
            nc.vector.reduce_sum(qbcol[:, 0:1], qt_sb[0], axis=AX)
            nc.vector.reduce_sum(qbcol[:, 1:2], qt_sb[1], axis=AX)
            nc.scalar.activation(awT[:, 0, :], qt_sb[2], Copy,
                                 accum_out=qbcol[:, 2:3])
            nc.scalar.activation(awT[:, 1, :], qt_sb[3], Copy,
                                 accum_out=qbcol[:, 3:4])
            qb16 = small.tile([P, DK], f16, tag="qb16")
            nc.scalar.copy(qb16, qbcol)

            # ---- g2col = (wqk^T @ qbar) column chunks [128, 4] ----
            g2c16 = small.tile([P, DK], f16, tag="g2c16")
            for m in range(DK):
                psg = ps_mm.tile([P, D], fp32, tag="mm")
                for kk in range(DK):
                    nc.tensor.matmul(
                        psg[:, 0:1], wqk_sb[:, kk, m * P:(m + 1) * P],
                        qb16[:, kk:kk + 1],
                        start=(kk == 0), stop=(kk == DK - 1),
                    )
                nc.scalar.copy(g2c16[:, m:m + 1], psg[:, 0:1])

            # ---- mc row [1, 2048] = g2 @ keys^T ----
            mc_flat = small.tile([1, L], fp32, tag="mc_flat")
            for nch in range(4):
                psm = ps_fe.tile([1, 512], fp32, tag="mc")
                for dk in range(DK):
                    nc.tensor.matmul(
                        psm, g2c16[:, dk:dk + 1],
                        kt_sb[:, dk, nch * 512:(nch + 1) * 512],
                        start=(dk == 0), stop=(dk == DK - 1),
                    )
                nc.scalar.copy(mc_flat[0:1, nch * 512:(nch + 1) * 512], psm)

            # ---- top-8 (DVE) + EXP (ACT) ----
            mx8 = small.tile([1, 8], fp32, tag="mx8")
            mi8 = small.tile([1, 8], u32, tag="mi8")
            nc.vector.max(out=mx8, in_=mc_flat)
            nc.vector.max_index(out=mi8, in_max=mx8, in_values=mc_flat)
            e4 = small.tile([1, K_SEL], fp32, tag="e4")
            nc.scalar.activation(e4, mx8[0:1, 0:K_SEL], Exp, scale=float(SCALE))

            # ---- VpT = wvh^T @ vt, lc-major to chase DMA chunks ----
            vpT = big.tile([P, 2, 2 * L], f16, tag="vpT", name="vpT")
            for lc in range(4):
                for ct in range(2):
                    pv = ps_mm.tile([P, 512], fp32, tag="mm")
                    for dk in range(DK):
                        nc.tensor.matmul(
                            pv,
                            wvh_sb[:, dk, ct * P:(ct + 1) * P],
                            vtq[lc][:, dk],
                            start=(dk == 0), stop=(dk == DK - 1),
                        )
                    nc.scalar.copy(vpT[:, ct, lc * 512:(lc + 1) * 512], pv)

            # ---- softmax tail + doubling on DVE (after find in FIFO) ----
            s1 = small.tile([1, 1], fp32, tag="s1")
            nc.vector.reduce_sum(s1, e4, axis=AX)
            r1 = small.tile([1, 1], fp32, tag="r1")
            nc.vector.reciprocal(r1, s1)
            w4 = small.tile([1, K_SEL], fp32, tag="w4")
            nc.vector.tensor_scalar(w4, e4, r1[0:1, 0:1], None, op0=MUL)
            nc.vector.tensor_copy(vpT[:, 0, L:2 * L], vpT[:, 0, 0:L])
            nc.vector.tensor_copy(vpT[:, 1, L:2 * L], vpT[:, 1, 0:L])

            wb = small.tile([P, K_SEL], fp32, tag="wb_sb")
            nc.gpsimd.partition_broadcast(wb, w4)
            wjI = [small.tile([P, P], f16, tag=f"wjI{j}", name=f"wjI{j}")
                   for j in range(K_SEL)]
            for j in range(K_SEL):
                nc.scalar.activation(wjI[j], ident16, Copy, scale=wb[:, j:j + 1])

            s_regs = [
                nc.values_load(
                    mi8[0:1, j:j + 1].bitcast(i32),
                    engines=(mybir.EngineType.PE,),
                    min_val=0, max_val=L - 1,
                    skip_runtime_bounds_check=True,
                ) for j in range(K_SEL)
            ]

            # ---- per (r, ct): AwT group -> 4 transposes into one bank ->
            #      one copy -> final GEMM -> out ----
            unit = 0
            for r in range(4):
                for ct in range(2):
                    pa = ps_mm.tile([P, 512], fp32, tag="mm")
                    for j in range(K_SEL):
                        nc.tensor.matmul(
                            pa, wjI[j],
                            vpT[:, ct, bass.ds(s_regs[j] + r * 512, 512)],
                            start=(j == 0), stop=(j == K_SEL - 1),
                        )
                    awTs = awT[:, ct, r * 512:(r + 1) * 512]
                    if unit % 2 == 0:
                        nc.scalar.copy(awTs, pa)
                    else:
                        nc.vector.tensor_copy(awTs, pa)
                    pt = ps_tp.tile([P, 512], f16, tag="tp")
                    for lp in range(4):
                        nc.tensor.transpose(
                            pt[:, lp * P:(lp + 1) * P],
                            awTs[:, lp * P:(lp + 1) * P], ident16)
                    aw = small.tile([P, 512], f16, tag=f"aw{unit % 3}",
                                    name=f"aw{r}_{ct}", bufs=1)
                    if unit % 2 == 0:
                        nc.vector.tensor_copy(aw, pt)
                    else:
                        nc.scalar.copy(aw, pt)
                    po = ps_mm.tile([P, D], fp32, tag="mm")
                    for lp in range(4):
                        nc.tensor.matmul(
                            po, aw[:, lp * P:(lp + 1) * P], wo_sb[:, lp],
                            start=(lp == 0), stop=(lp == DK - 1),
                        )
                    ot = stream.tile([P, D], fp32, tag="otile")
                    if unit % 2 == 0:
                        nc.scalar.copy(ot, po)
                    else:
                        nc.vector.tensor_copy(ot, po)
                    row0 = r * 256 + ct * P
                    eng = nc.sync if ct == 0 else nc.scalar
                    eng.dma_start(out_dram[row0:row0 + P, :], ot)
                    unit += 1

    nc.compile()
    return nc


_NC_CACHE = None


def _get_nc():
    global _NC_CACHE
    if _NC_CACHE is None:
        _NC_CACHE = _build_nc()
    return _NC_CACHE


def _half_cols(half):
    d0 = 32 * half
    return np.array([(cl // 32) * 64 + d0 + cl % 32 for cl in range(CH)])


def _row_index(half):
    # device row r*256 + cl  ->  full-output row i
    d0 = 32 * half
    idx = np.empty(1024, np.int64)
    for r in range(4):
        for cl in range(CH):
            i = (d0 + cl % 32) * 32 + (cl // 32) * 4 + r
            idx[r * CH + cl] = i
    return idx


def make_in_maps(queries, keys, values, wq, wk, wv, wo):
    import ml_dtypes
    f8 = ml_dtypes.float8_e4m3
    wqk = (wq.astype(np.float64) @ wk.T.astype(np.float64)).astype(np.float16)
    wo16 = wo.astype(np.float16)
    in_maps = []
    for c in range(N_CORES):
        b, half = c // 2, c % 2
        qt = np.ascontiguousarray(queries[b].T).astype(np.float16)
        kt = np.ascontiguousarray(keys[b].T).astype(f8)
        vt = np.ascontiguousarray(values[b].T).astype(np.float16)
        wvh = np.ascontiguousarray(wv[:, _half_cols(half)]).astype(np.float16)
        in_maps.append({
            "qt": qt, "kt": kt, "vt": vt,
            "wqk": wqk, "wvh": wvh, "wo": wo16,
        })
    return in_maps


def kernel(queries, keys, values, wq, wk, wv, wo, trace=False):
    import sys
    if "/opt/trn_rl_repo" not in sys.path:
        sys.path.insert(0, "/opt/trn_rl_repo")
    from concourse import bass_utils

    nc = _get_nc()
    in_maps = make_in_maps(queries, keys, values, wq, wk, wv, wo)
    res = bass_utils.run_bass_kernel_spmd(
        nc, in_maps, core_ids=list(range(N_CORES)), trace=trace,
    )
    out = np.empty((B, L, D), np.float32)
    for c in range(N_CORES):
        b, half = c // 2, c % 2
        out[b, _row_index(half), :] = res.results[c]["out"]
    if trace:
        return out, res
    return out


# revision 12
# speedup vs baseline: 1.2857x; 1.2432x over previous
"""AutoCorrelation Trainium2 kernel (v5).

Reference reformulation (verified to 3e-7 rel):
  H=8, L=2048, D=512, k_sel=4, SCALE=1/(H*L)
  qbar = sum_l queries[b,l,:]
  mc = qbar @ wqk @ keys^T           (wqk = wq @ wk^T, host-precomputed)
  top_idx = top4(mc); w = softmax(SCALE * top_vals)
  VpT[c, l] = (values[b] @ wv_half)^T
  AwT[c, l] = sum_j w_j VpT[c, (l + idx_j) mod L]
  out rows (reference transpose quirk): for each r in 0..3:
      out_rows(r) = Aw[r*512:(r+1)*512, :].T @ wo

Sharding: 8 cores = 4 batches x 2 channel-halves; per-batch front-end
computed redundantly on the half-pair.

v5 (per core):
  - 3 DMA issuers: sync HWDGE (qt01/wqk/vtAB), ACT HWDGE (qt23/wvh/vtC),
    GpSimd SWDGE (kt/vtD/wo) -- keeps ACT free for compute after 4 issues
  - small PE warm chain; qbar split DVE reduces + ACT accum_out
  - g2col, mc (fp8 kt), top-8 DVE, softmax, GpSimd w-broadcast, ACT w-identities
  - VpT chases vt l-chunk DMAs; doubling via 2 ACT copies
  - AwT per r: ldweights(wjI_j) once + 2 no-reload matmuls (ct0, ct1) with
    dynamic-register rhs slices of doubled VpT
  - transposes via dma_start_transpose ([128,512] -> [128,4,128] blockwise,
    one call per unit on sync/ACT rings) -- no PE transposes, no extra PSUM
  - final GEMM Aw_r^T @ wo per unit, fp32 out
Dtypes: q fp16, k fp8e4m3, v/weights/intermediates fp16, accum fp32.
"""

import numpy as np

B, L, D = 4, 2048, 512
H = 8
K_SEL = 4
SCALE = 1.0 / (H * L)
N_CORES = 8
P = 128
CH = 256          # channels per core (half of 512)
DK = D // P       # 4 d-tiles
N_WARM = 12


def _build_nc():
    import concourse.bass as bass
    import concourse.bacc as bacc
    import concourse.mybir as mybir
    from concourse.tile import TileContext
    from concourse.masks import make_identity

    fp32 = mybir.dt.float32
    f16 = mybir.dt.float16
    f8 = mybir.dt.float8e4
    u32 = mybir.dt.uint32
    i32 = mybir.dt.int32
    AX = mybir.AxisListType.X
    MUL = mybir.AluOpType.mult
    Copy = mybir.ActivationFunctionType.Copy
    Exp = mybir.ActivationFunctionType.Exp

    nc = bacc.Bacc("TRN2", target_bir_lowering=False, debug=False, num_devices=N_CORES)

    qt_dram = nc.dram_tensor("qt", [D, L], f16, kind="ExternalInput")    # queries^T
    kt_dram = nc.dram_tensor("kt", [D, L], f8, kind="ExternalInput")     # keys^T
    vt_dram = nc.dram_tensor("vt", [D, L], f16, kind="ExternalInput")    # values^T
    wqk_dram = nc.dram_tensor("wqk", [D, D], f16, kind="ExternalInput")  # wq @ wk^T
    wvh_dram = nc.dram_tensor("wvh", [D, CH], f16, kind="ExternalInput")
    wo_dram = nc.dram_tensor("wo", [D, D], f16, kind="ExternalInput")
    out_dram = nc.dram_tensor("out", [L // 2, D], fp32, kind="ExternalOutput")

    with TileContext(nc) as tc:
        with (
            tc.tile_pool(name="const", bufs=1) as cpool,
            tc.tile_pool(name="wts", bufs=1) as wts,
            tc.tile_pool(name="big", bufs=1) as big,
            tc.tile_pool(name="stream", bufs=2) as stream,
            tc.tile_pool(name="small", bufs=1) as small,
            tc.tile_pool(name="ps_warm", bufs=1, space="PSUM") as ps_warm,
            tc.tile_pool(name="ps_fe", bufs=2, space="PSUM") as ps_fe,
            tc.tile_pool(name="ps_mm", bufs=4, space="PSUM") as ps_mm,
        ):
            ident = cpool.tile([P, P], fp32, tag="ident")
            make_identity(nc, ident)
            ident16 = cpool.tile([P, P], f16, tag="ident16")
            nc.scalar.copy(ident16, ident)
            zscr = cpool.tile([P, D], f16, tag="zscr")
            nc.gpsimd.memset(zscr, 0.0)

            # ---- small PE warm-up chain ----
            warm = ps_warm.tile([P, D], fp32, tag="warm")
            for i in range(N_WARM):
                nc.tensor.matmul(warm, ident16, zscr,
                                 start=(i == 0), stop=(i == N_WARM - 1))

            # ---- SBUF input tiles ----
            qt_sb = [big.tile([P, L], f16, tag=f"qt{i}", name=f"qt{i}")
                     for i in range(DK)]
            kt_sb = big.tile([P, DK, L], f8, tag="kt", name="kt")
            vtq = [big.tile([P, DK, 512], f16, tag=f"vt{i}", name=f"vt{i}")
                   for i in range(4)]
            wqk_sb = wts.tile([P, DK, D], f16, tag="wqk", name="wqk")
            wvh_sb = wts.tile([P, DK, CH], f16, tag="wvh", name="wvh")
            wo_sb = wts.tile([P, DK, D], f16, tag="wo", name="wo")

            qt_v = qt_dram.rearrange("(t p) l -> p t l", p=P)
            kt_v = kt_dram.rearrange("(t p) l -> p t l", p=P)
            vt_v = vt_dram.rearrange("(t p) l -> p t l", p=P)
            wqk_v = wqk_dram.rearrange("(t p) d -> p t d", p=P)
            wvh_v = wvh_dram.rearrange("(t p) c -> p t c", p=P)
            wo_v = wo_dram.rearrange("(t p) d -> p t d", p=P)

            # sync: qt0 qt1 wqk vtA vtB | ACT: qt2 qt3 wvh vtC | gpsimd: kt vtD wo
            nc.sync.dma_start(qt_sb[0], qt_v[:, 0])
            nc.scalar.dma_start(qt_sb[2], qt_v[:, 2])
            nc.gpsimd.dma_start(kt_sb, kt_v)
            nc.sync.dma_start(qt_sb[1], qt_v[:, 1])
            nc.scalar.dma_start(qt_sb[3], qt_v[:, 3])
            nc.sync.dma_start(wqk_sb, wqk_v)
            nc.scalar.dma_start(wvh_sb, wvh_v)
            nc.sync.dma_start(vtq[0], vt_v[:, :, 0:512])
            nc.scalar.dma_start(vtq[2], vt_v[:, :, 1024:1536])
            nc.gpsimd.dma_start(vtq[3], vt_v[:, :, 1536:2048])
            nc.sync.dma_start(vtq[1], vt_v[:, :, 512:1024])
            nc.gpsimd.dma_start(wo_sb, wo_v)

            # ---- qbar: 2 DVE reduces + 2 ACT accum activations ----
            awT = big.tile([P, 2, L], f16, tag="awT", name="awT")
            qbcol = small.tile([P, DK], fp32, tag="qbcol")
            nc.vector.reduce_sum(qbcol[:, 0:1], qt_sb[0], axis=AX)
            nc.vector.reduce_sum(qbcol[:, 1:2], qt_sb[1], axis=AX)
            nc.scalar.activation(awT[:, 0, :], qt_sb[2], Copy,
                                 accum_out=qbcol[:, 2:3])
            nc.scalar.activation(awT[:, 1, :], qt_sb[3], Copy,
                                 accum_out=qbcol[:, 3:4])
            qb16 = small.tile([P, DK], f16, tag="qb16")
            nc.scalar.copy(qb16, qbcol)

            # ---- g2col = (wqk^T @ qbar) column chunks [128, 4] ----
            g2c16 = small.tile([P, DK], f16, tag="g2c16")
            for m in range(DK):
                psg = ps_mm.tile([P, D], fp32, tag="mm")
                for kk in range(DK):
                    nc.tensor.matmul(
                        psg[:, 0:1], wqk_sb[:, kk, m * P:(m + 1) * P],
                        qb16[:, kk:kk + 1],
                        start=(kk == 0), stop=(kk == DK - 1),
                    )
                nc.scalar.copy(g2c16[:, m:m + 1], psg[:, 0:1])

            # ---- mc row [1, 2048] = g2 @ keys^T ----
            mc_flat = small.tile([1, L], fp32, tag="mc_flat")
            for nch in range(4):
                psm = ps_fe.tile([1, 512], fp32, tag="mc")
                for dk in range(DK):
                    nc.tensor.matmul(
                        psm, g2c16[:, dk:dk + 1],
                        kt_sb[:, dk, nch * 512:(nch + 1) * 512],
                        start=(dk == 0), stop=(dk == DK - 1),
                    )
                nc.scalar.copy(mc_flat[0:1, nch * 512:(nch + 1) * 512], psm)

            # ---- top-8 on DVE ----
            mx8 = small.tile([1, 8], fp32, tag="mx8")
            mi8 = small.tile([1, 8], u32, tag="mi8")
            nc.vector.max(out=mx8, in_=mc_flat)
            nc.vector.max_index(out=mi8, in_max=mx8, in_values=mc_flat)

            # ---- VpT = wvh^T @ vt, lc-major to chase DMA chunks ----
            vpT = big.tile([P, 2, 2 * L], f16, tag="vpT", name="vpT")
            for lc in range(4):
                for ct in range(2):
                    pv = ps_mm.tile([P, 512], fp32, tag="mm")
                    for dk in range(DK):
                        nc.tensor.matmul(
                            pv,
                            wvh_sb[:, dk, ct * P:(ct + 1) * P],
                            vtq[lc][:, dk],
                            start=(dk == 0), stop=(dk == DK - 1),
                        )
                    nc.scalar.copy(vpT[:, ct, lc * 512:(lc + 1) * 512], pv)
            # doubling on ACT (after VpT copies in ACT FIFO)
            nc.scalar.copy(vpT[:, 0, L:2 * L], vpT[:, 0, 0:L])
            nc.scalar.copy(vpT[:, 1, L:2 * L], vpT[:, 1, 0:L])

            # ---- softmax tail ----
            e4 = small.tile([1, K_SEL], fp32, tag="e4")
            nc.scalar.activation(e4, mx8[0:1, 0:K_SEL], Exp, scale=float(SCALE))
            s1 = small.tile([1, 1], fp32, tag="s1")
            nc.vector.reduce_sum(s1, e4, axis=AX)
            r1 = small.tile([1, 1], fp32, tag="r1")
            nc.vector.reciprocal(r1, s1)
            w4 = small.tile([1, K_SEL], fp32, tag="w4")
            nc.vector.tensor_scalar(w4, e4, r1[0:1, 0:1], None, op0=MUL)
            wb = small.tile([P, K_SEL], fp32, tag="wb_sb")
            nc.gpsimd.partition_broadcast(wb, w4)
            wjI = [small.tile([P, P], f16, tag=f"wjI{j}", name=f"wjI{j}")
                   for j in range(K_SEL)]
            for j in range(K_SEL):
                nc.scalar.activation(wjI[j], ident16, Copy, scale=wb[:, j:j + 1])

            s_regs = [
                nc.values_load(
                    mi8[0:1, j:j + 1].bitcast(i32),
                    engines=(mybir.EngineType.PE,),
                    min_val=0, max_val=L - 1,
                    skip_runtime_bounds_check=True,
                ) for j in range(K_SEL)
            ]

            # ---- per r: paired AwT groups (ldweights reused across ct);
            #      per unit: copy -> DMA blockwise transpose -> final ----
            pas = {}
            aws = {}
            unit = 0

            def emit_awt(r):
                pa = [ps_mm.tile([P, 512], fp32, tag="mm", name=f"pa{r}_{c}")
                      for c in range(2)]
                for j in range(K_SEL):
                    for ct in range(2):
                        nc.tensor.matmul(
                            pa[ct], wjI[j],
                            vpT[:, ct, bass.ds(s_regs[j] + r * 512, 512)],
                            start=(j == 0), stop=(j == K_SEL - 1),
                            skip_group_check=True,
                        )
                pas[r] = pa

            def emit_tail(r, ct, u):
                pa = pas[r][ct]
                awTs = awT[:, ct, r * 512:(r + 1) * 512]
                if u % 2 == 0:
                    nc.scalar.copy(awTs, pa)
                else:
                    nc.vector.tensor_copy(awTs, pa)
                aw = small.tile([P, DK, P], f16, tag=f"aw{u % 4}",
                                name=f"aw{r}_{ct}")
                eng = nc.sync if ct == 0 else nc.scalar
                eng.dma_start_transpose(aw, awTs)
                aws[(r, ct)] = aw

            def emit_final(r, ct, u):
                aw = aws[(r, ct)]
                po = ps_mm.tile([P, D], fp32, tag="mm")
                for lp in range(4):
                    nc.tensor.matmul(
                        po, aw[:, lp], wo_sb[:, lp],
                        start=(lp == 0), stop=(lp == DK - 1),
                    )
                ot = stream.tile([P, D], fp32, tag="otile")
                if u % 2 == 0:
                    nc.scalar.copy(ot, po)
                else:
                    nc.vector.tensor_copy(ot, po)
                row0 = r * 256 + ct * P
                eng = nc.sync if ct == 0 else nc.scalar
                eng.dma_start(out_dram[row0:row0 + P, :], ot)

            # software-pipelined emission: AwT(r) ... final(r-1)
            emit_awt(0)
            emit_tail(0, 0, 0); emit_tail(0, 1, 1)
            emit_awt(1)
            emit_tail(1, 0, 2); emit_tail(1, 1, 3)
            emit_final(0, 0, 0); emit_final(0, 1, 1)
            emit_awt(2)
            emit_tail(2, 0, 4); emit_tail(2, 1, 5)
            emit_final(1, 0, 2); emit_final(1, 1, 3)
            emit_awt(3)
            emit_tail(3, 0, 6); emit_tail(3, 1, 7)
            emit_final(2, 0, 4); emit_final(2, 1, 5)
            emit_final(3, 0, 6); emit_final(3, 1, 7)

    nc.compile()
    return nc


_NC_CACHE = None


def _get_nc():
    global _NC_CACHE
    if _NC_CACHE is None:
        _NC_CACHE = _build_nc()
    return _NC_CACHE


def _half_cols(half):
    d0 = 32 * half
    return np.array([(cl // 32) * 64 + d0 + cl % 32 for cl in range(CH)])


def _row_index(half):
    # device row r*256 + cl  ->  full-output row i
    d0 = 32 * half
    idx = np.empty(1024, np.int64)
    for r in range(4):
        for cl in range(CH):
            i = (d0 + cl % 32) * 32 + (cl // 32) * 4 + r
            idx[r * CH + cl] = i
    return idx


def make_in_maps(queries, keys, values, wq, wk, wv, wo):
    import ml_dtypes
    f8 = ml_dtypes.float8_e4m3
    wqk = (wq.astype(np.float64) @ wk.T.astype(np.float64)).astype(np.float16)
    wo16 = wo.astype(np.float16)
    in_maps = []
    for c in range(N_CORES):
        b, half = c // 2, c % 2
        qt = np.ascontiguousarray(queries[b].T).astype(np.float16)
        kt = np.ascontiguousarray(keys[b].T).astype(f8)
        vt = np.ascontiguousarray(values[b].T).astype(np.float16)
        wvh = np.ascontiguousarray(wv[:, _half_cols(half)]).astype(np.float16)
        in_maps.append({
            "qt": qt, "kt": kt, "vt": vt,
            "wqk": wqk, "wvh": wvh, "wo": wo16,
        })
    return in_maps


def kernel(queries, keys, values, wq, wk, wv, wo, trace=False):
    import sys
    if "/opt/trn_rl_repo" not in sys.path:
        sys.path.insert(0, "/opt/trn_rl_repo")
    from concourse import bass_utils

    nc = _get_nc()
    in_maps = make_in_maps(queries, keys, values, wq, wk, wv, wo)
    res = bass_utils.run_bass_kernel_spmd(
        nc, in_maps, core_ids=list(range(N_CORES)), trace=trace,
    )
    out = np.empty((B, L, D), np.float32)
    for c in range(N_CORES):
        b, half = c // 2, c % 2
        out[b, _row_index(half), :] = res.results[c]["out"]
    if trace:
        return out, res
    return out
